# revision 20
# baseline (speedup 1.0000x reference)
"""Trainium2 Bass kernel for nn_CSMHSA (cross-scale multi-head self-attention).

Reference computation (per batch element b):
    q = conv1x1(upsample2x(x_high), Wq)        # [256, 32, 32]
    k = conv1x1(x_low, Wk)                     # [256, 32, 32]
    v = conv1x1(x_low, Wv)                     # [256, 32, 32]
    per head h (8 heads, d=32): scores = q_h^T k_h  -> softmax over j -> out = v_h @ attn^T

Key algebraic optimization: the 2x nearest-neighbor upsample happens BEFORE the
pointwise conv, so q has only 256 unique columns (the 16x16 coarse grid).
Attention is therefore computed at coarse resolution (i in [0,256)) and the
final 2x upsample is a pure data-movement fused into the output write.
This cuts score/AV matmul and softmax-exp work by 4x.

Layout trick: scores are computed TRANSPOSED, scoresT[j, i] (j on partitions),
so that
  - exp(scoresT) (ScalarE, PSUM->SBUF) directly produces the AV matmul's
    moving operand (no transpose of the 2M-element attention matrix),
  - the softmax denominator Z[i] = sum_j E[j, i] is a partition-axis sum,
    obtained for free with a ones-vector stationary operand on the PE.
Normalization by 1/Z is folded into the final upsample write.

Sharding: pure data-parallel over batch: core b processes batch element b.
Biases bq/bk/bv are zeros by problem construction (spec fill: zeros);
additionally, a k-bias provably cannot change the output (it shifts each
softmax row by a constant), so only q/v biases would matter -- both zero here.
"""

import sys

import numpy as np

for _p in ("/opt/trn_rl_repo",):
    if _p not in sys.path:
        sys.path.insert(0, _p)

P = 128
CH = 512  # x_high channels
C = 256  # attention channels
S = 1024  # 32*32 low-res spatial
SC = 256  # 16*16 coarse spatial
NHEADS = 8
D = 32

# head m (within a 128-channel group) -> column offset of its [*,256] block
# inside the [128, 1024] E tiles.
ECOLS = (0, 256, 512, 768)

_CACHE = {}


def _emit(nc, tile, mybir):
    f32 = mybir.dt.float32
    AF = mybir.ActivationFunctionType

    xh = nc.dram_tensor("xh", [CH, SC], f32, kind="ExternalInput")
    xl = nc.dram_tensor("xl", [C, S], f32, kind="ExternalInput")
    wqT = nc.dram_tensor("wqT", [CH, C], f32, kind="ExternalInput")
    wkT = nc.dram_tensor("wkT", [C, C], f32, kind="ExternalInput")
    wvT = nc.dram_tensor("wvT", [C, C], f32, kind="ExternalInput")
    out = nc.dram_tensor("out", [C, S], f32, kind="ExternalOutput")

    with tile.TileContext(nc) as tc:
        with (
            tc.tile_pool(name="consts", bufs=1) as consts,
            tc.tile_pool(name="work", bufs=1) as work,
            tc.tile_pool(name="epool", bufs=3) as epool,
            tc.tile_pool(name="psum", bufs=3, space="PSUM") as psum,
            tc.tile_pool(name="avpool", bufs=2, space="PSUM") as avpool,
        ):
            # ---- input DMAs ----
            xh_sb = consts.tile([P, 4, SC], f32)
            nc.sync.dma_start(xh_sb, xh[:, :].rearrange("(kc p) s -> p kc s", p=P))
            wqT_sb = consts.tile([P, 4, C], f32)
            nc.sync.dma_start(wqT_sb, wqT[:, :].rearrange("(kc p) c -> p kc c", p=P))
            xl_sb = consts.tile([P, 2, S], f32)
            nc.sync.dma_start(xl_sb, xl[:, :].rearrange("(kc p) s -> p kc s", p=P))
            wkT_sb = consts.tile([P, 2, C], f32)
            nc.sync.dma_start(wkT_sb, wkT[:, :].rearrange("(kc p) c -> p kc c", p=P))
            wvT_sb = consts.tile([P, 2, C], f32)
            nc.sync.dma_start(wvT_sb, wvT[:, :].rearrange("(kc p) c -> p kc c", p=P))
            ones_sb = consts.tile([P, 32], f32)
            nc.vector.memset(ones_sb, 1.0)

            # Warm the ScalarE exp table set early so the ~2.7us table load
            # happens during the input DMAs, not on the first real exp.
            warm_sb = work.tile([1, 1], f32)
            nc.scalar.activation(warm_sb, ones_sb[0:1, 0:1], AF.Exp)

            qs_sb = work.tile([P, 2, SC], f32)
            k_sb = work.tile([P, 2, S], f32)
            vT_sb = work.tile([P, 8, C], f32)
            rz_sb = work.tile([P, 2, SC], f32)
            out_sb = work.tile([P, 2, S], f32)

            # ---- projections ----
            # qs[c, i] = sum_ch Wq[c, ch] xh[ch, i]   (coarse-grid q)
            for g in range(2):
                qp = psum.tile([P, SC], f32, tag="big", name=f"qp{g}")
                for kc in range(4):
                    nc.tensor.matmul(
                        qp,
                        wqT_sb[:, kc, P * g : P * (g + 1)],
                        xh_sb[:, kc, :],
                        start=(kc == 0),
                        stop=(kc == 3),
                    )
                nc.vector.tensor_copy(qs_sb[:, g, :], qp)
            # k[c, j] = sum_c' Wk[c, c'] xl[c', j]
            for g in range(2):
                kp = psum.tile([P, S], f32, tag="big", name=f"kp{g}")
                for kc in range(2):
                    for nh in range(2):
                        nc.tensor.matmul(
                            kp[:, 512 * nh : 512 * (nh + 1)],
                            wkT_sb[:, kc, P * g : P * (g + 1)],
                            xl_sb[:, kc, 512 * nh : 512 * (nh + 1)],
                            start=(kc == 0),
                            stop=(kc == 1),
                        )
                nc.vector.tensor_copy(k_sb[:, g, :], kp)
            # vT[j, c] = sum_c' xl[c', j] Wv[c, c']   (v produced pre-transposed)
            for q4 in range(2):
                vp = psum.tile([P, S], f32, tag="big", name=f"vp{q4}")
                for t in range(4):
                    jc = 4 * q4 + t
                    for kc in range(2):
                        nc.tensor.matmul(
                            vp[:, 256 * t : 256 * (t + 1)],
                            xl_sb[:, kc, P * jc : P * (jc + 1)],
                            wvT_sb[:, kc, :],
                            start=(kc == 0),
                            stop=(kc == 1),
                        )
                nc.vector.tensor_copy(
                    vT_sb[:, 4 * q4 : 4 * q4 + 4, :],
                    vp.rearrange("p (t c) -> p t c", t=4),
                )

            # ---- attention (channel group g holds heads 4g..4g+3) ----
            av = []
            for g in range(2):
                avp = avpool.tile([P, 2 * SC], f32, tag="av", name=f"av{g}")
                av.append(avp)
                for jc in range(8):
                    # scoresT[j, i] for the 4 heads of this group, 4-way
                    # row-tiled (K=32 each): head m reads SBUF partitions 32m.
                    # Concurrent row-tiled matmuls MUST drain to DISTINCT PSUM
                    # banks (same-bank concurrent PE writes are a fatal HW
                    # collision), so the 4 heads go to 4 banks across two
                    # 2-bank tiles: m0/m1 -> spa banks 0/1, m2/m3 -> spb.
                    spa = psum.tile([P, S], f32, tag="big", name=f"spa{g}_{jc}")
                    spb = psum.tile([P, S], f32, tag="big", name=f"spb{g}_{jc}")
                    for m in range(4):
                        sp = spa if m < 2 else spb
                        nc.tensor.matmul(
                            sp[:, 512 * (m % 2) : 512 * (m % 2) + SC],
                            k_sb[32 * m : 32 * (m + 1), g, P * jc : P * (jc + 1)],
                            qs_sb[32 * m : 32 * (m + 1), g, :],
                            start=True,
                            stop=True,
                            tile_position=(32 * m, 0),
                        )
                    e_sb = epool.tile([P, S], f32, tag="E", name=f"e{g}_{jc}")
                    for half, sp in ((0, spa), (1, spb)):
                        nc.scalar.activation(
                            e_sb[:, 512 * half : 512 * (half + 1)].rearrange(
                                "p (b x) -> p b x", b=2
                            ),
                            sp.rearrange("p (b x) -> p b x", b=2)[:, :, 0:SC],
                            AF.Exp,
                        )
                    # AV: out[d, i] += vT[j, d]^T E[j, i], 4-way column-tiled.
                    for m in range(4):
                        nc.tensor.matmul(
                            avp[32 * m : 32 * (m + 1), 0:SC],
                            vT_sb[:, jc, P * g + 32 * m : P * g + 32 * (m + 1)],
                            e_sb[:, ECOLS[m] : ECOLS[m] + SC],
                            start=(jc == 0),
                            stop=(jc == 7),
                            tile_position=(0, 32 * m),
                            skip_group_check=True,
                        )
                    # Z[i] += sum_j E[j, i], replicated into all 32 partitions
                    # of the head's group via an all-ones [128, 32] stationary
                    # operand -- this doubles as the 1/Z broadcast layout.
                    # start is never set: AV-m's start at jc==0 already cleared
                    # has_written for these partitions' whole bank (2KB zero
                    # region), so Z's first write overwrites correctly.
                    for m in range(4):
                        nc.tensor.matmul(
                            avp[32 * m : 32 * (m + 1), SC : 2 * SC],
                            ones_sb,
                            e_sb[:, ECOLS[m] : ECOLS[m] + SC],
                            start=False,
                            stop=(jc == 7),
                            tile_position=(0, 32 * m),
                            skip_group_check=True,
                        )

                # ---- per-group endgame (overlaps with the other group) ----
                # Z is already replicated per channel row; one reciprocal gives
                # the fully-broadcast 1/Z[head(c), i] directly.
                nc.vector.reciprocal_approx_fast(rz_sb[:, g, :], avp[:, SC : 2 * SC])
                # Fused normalize + 2x nearest-neighbor upsample:
                # out[c, (2yc+dy)*32 + 2xc+dx] = av[c, yc*16+xc] * rz[c, yc*16+xc]
                avv = avp[:, 0:SC].rearrange("p (yc xc) -> p yc xc", yc=16)
                rzv = rz_sb[:, g, :].rearrange("p (yc xc) -> p yc xc", yc=16)
                ov = out_sb[:, g, :].rearrange(
                    "p (yc dy xc dx) -> p yc dy xc dx", dy=2, dx=2, xc=16
                )
                for dy in range(2):
                    nc.vector.tensor_mul(
                        ov[:, :, dy, :, :],
                        avv[:, :, :, None].to_broadcast((P, 16, 16, 2)),
                        rzv[:, :, :, None].to_broadcast((P, 16, 16, 2)),
                    )
                nc.sync.dma_start(out[P * g : P * (g + 1), :], out_sb[:, g, :])

    return nc


def _get_nc():
    if "nc" not in _CACHE:
        import concourse.bacc as bacc
        import concourse.tile as tile
        from concourse import mybir

        # Bacc (not raw Bass): its compile pipeline moves excess matmul waits
        # onto ldweights and splits multi-wait sync into event semaphores,
        # which the TRN2 PE instruction format requires (max 1 wait/inst).
        nc = bacc.Bacc("TRN2")
        _emit(nc, tile, mybir)
        nc.compile()
        _CACHE["nc"] = nc
    return _CACHE["nc"]


def _make_in_maps(x_high, x_low, Wq, Wk, Wv):
    B = x_high.shape[0]
    wqT = np.ascontiguousarray(np.asarray(Wq, np.float32).T)
    wkT = np.ascontiguousarray(np.asarray(Wk, np.float32).T)
    wvT = np.ascontiguousarray(np.asarray(Wv, np.float32).T)
    in_maps = []
    for b in range(B):
        in_maps.append(
            {
                "xh": np.ascontiguousarray(
                    np.asarray(x_high[b], np.float32).reshape(CH, SC)
                ),
                "xl": np.ascontiguousarray(
                    np.asarray(x_low[b], np.float32).reshape(C, S)
                ),
                "wqT": wqT,
                "wkT": wkT,
                "wvT": wvT,
            }
        )
    return in_maps


def kernel(x_high, x_low, Wq, bq, Wk, bk, Wv, bv):
    """Full-input entry point: shards batch over 8 NeuronCores, returns the
    full [8, 256, 32, 32] float32 output. bq/bk/bv are zeros by problem spec
    (and a k-bias cannot affect the output at all); they are not applied."""
    from concourse.bass_utils import run_bass_kernel_spmd

    x_high = np.asarray(x_high)
    B = x_high.shape[0]
    nc = _get_nc()
    in_maps = _make_in_maps(x_high, np.asarray(x_low), Wq, Wk, Wv)
    res = run_bass_kernel_spmd(nc, in_maps, core_ids=list(range(B)))
    out = np.stack([r["out"].reshape(C, 32, 32) for r in res.results], axis=0)
    return out.astype(np.float32)
